# revision 10
# baseline (speedup 1.0000x reference)
"""Trainium2 Bass kernel for nn_MihGNNEmbeddingTest3 (gnn_message_passing).

Reference math:
    H = mlp(A_s @ emb)          (mlp = 3 linear layers, no activations)
    out[e] = relu(|<H[src_e], H[dst_e]>| / (||H[src_e]|| ||H[dst_e]||))

Since the mlp is affine, fold it:  H = A_s @ (emb @ W_eff^T) + b_eff.
Device work per core (node-sharded):  H_c = A_s[rows_c] @ E2 + b_eff
(E2 = emb @ W_eff^T precomputed on host), AllGather H, then per-edge
indirect row gathers + fused dot/norm reductions.

Sharding: A_s rows (and nodes) split 1024/core across 8 cores; edges
split 1024/core. A_s shard is shipped pre-transposed in bf16 so k-tiles
land directly as matmul lhsT weights.
"""

import os
import sys

import numpy as np

try:
    import concourse.bass  # noqa: F401
except ImportError:  # pragma: no cover - grading env should have PYTHONPATH set
    for p in ("/opt/trn_rl_repo", "/root/.axon_site/_ro/trn_rl_repo"):
        if os.path.isdir(p) and p not in sys.path:
            sys.path.insert(0, p)

import ml_dtypes

N, D, B = 8192, 256, 8192
N_CORES = 8
ROWS = N // N_CORES  # A_s rows / nodes per core
EPC = B // N_CORES   # edges per core
KT = N // 128        # contraction tiles
MT = ROWS // 128     # output row tiles per core
JT = EPC // 128      # edge blocks per core

_CACHE = {}
LAST_RESULTS = None  # BassKernelResults of the most recent run (for test.py)


def _build():
    import concourse.bacc as bacc
    import concourse.bass as bass
    import concourse.mybir as mybir
    import concourse.tile as tile

    fp32 = mybir.dt.float32
    bf16 = mybir.dt.bfloat16

    nc = bacc.Bacc(num_devices=N_CORES)
    # partition-major layouts: [p, k_tile, cols] so each DMA chunk reads
    # large contiguous per-partition spans from DRAM
    at = nc.declare_dram_parameter("at", [128, KT, ROWS], bf16, isOutput=False)
    e2 = nc.declare_dram_parameter("e2", [128, KT, D], bf16, isOutput=False)
    bias = nc.declare_dram_parameter("bias", [128, D], fp32, isOutput=False)
    sidx = nc.declare_dram_parameter("sidx", [128, JT], mybir.dt.int32, isOutput=False)
    didx = nc.declare_dram_parameter("didx", [128, JT], mybir.dt.int32, isOutput=False)
    out = nc.declare_dram_parameter("out", [128, JT], fp32, isOutput=True)

    with tile.TileContext(nc) as tc:
        with (
            tc.tile_pool(name="atp", bufs=8) as atp,
            tc.tile_pool(name="e2p", bufs=4) as e2p,
            tc.tile_pool(name="psum", bufs=MT, space="PSUM") as psum,
            tc.tile_pool(name="hsb", bufs=4) as hsbp,
            tc.tile_pool(name="dram", bufs=1, space="DRAM") as dram,
            tc.tile_pool(name="const", bufs=1) as constp,
            tc.tile_pool(name="gat", bufs=3) as gat,
            tc.tile_pool(name="small", bufs=3) as small,
        ):
            h_shard = dram.tile([ROWS, D], bf16)
            h_full = dram.tile([N, D], bf16, addr_space="Shared")

            # Batched loads: few big DMAs with 8-16KB contiguous descriptors
            # (per-dma_start issue overhead was pacing the whole matmul).
            AT_CH, E2_CH = 8, 16  # k-tiles per DMA chunk
            at_c = [None] * (KT // AT_CH)
            e2_c = [None] * (KT // E2_CH)

            def load_e2(c):
                ec = e2p.tile([128, E2_CH, D], bf16, name=f"e2c_{c}", tag="e2c")
                nc.sync.dma_start(out=ec[:], in_=e2[:, c * E2_CH:(c + 1) * E2_CH, :])
                e2_c[c] = ec

            def load_at(c):
                ac = atp.tile([128, AT_CH, ROWS], bf16, name=f"atc_{c}", tag="atc")
                nc.sync.dma_start(out=ac[:], in_=at[:, c * AT_CH:(c + 1) * AT_CH, :])
                at_c[c] = ac

            # interleave so k-order data arrives first
            load_e2(0)
            for c in range(KT // AT_CH):
                load_at(c)
                if c % 2 == 1 and c // 2 + 1 < KT // E2_CH:
                    load_e2(c // 2 + 1)
            at_t = [at_c[k // AT_CH][:, k % AT_CH, :] for k in range(KT)]
            e2_t = [e2_c[k // E2_CH][:, k % E2_CH, :] for k in range(KT)]

            bias_sb = constp.tile([128, D], fp32)
            nc.sync.dma_start(out=bias_sb[:], in_=bias[:])
            sidx_sb = constp.tile([128, JT], mybir.dt.int32)
            nc.sync.dma_start(out=sidx_sb[:], in_=sidx[:])
            didx_sb = constp.tile([128, JT], mybir.dt.int32)
            nc.sync.dma_start(out=didx_sb[:], in_=didx[:])
            out_sb = constp.tile([128, JT], fp32)

            with nc.named_scope("matmul"):
                # k-outer: every AT k-tile feeds all MT psum accumulators the
                # moment it lands, so PE keeps pace with the streaming DMA.
                ps_t = [
                    psum.tile([128, D], fp32, name=f"ps_{m}", tag="ps")
                    for m in range(MT)
                ]
                for k in range(KT):
                    for m in range(MT):
                        nc.tensor.matmul(
                            out=ps_t[m][:],
                            lhsT=at_t[k][:, m * 128:(m + 1) * 128],
                            rhs=e2_t[k],
                            start=(k == 0),
                            stop=(k == KT - 1),
                        )
                for m in range(MT):
                    hs = hsbp.tile([128, D], bf16, name=f"h_{m}", tag="h")
                    nc.vector.tensor_tensor(
                        out=hs[:], in0=ps_t[m][:], in1=bias_sb[:],
                        op=mybir.AluOpType.add,
                    )
                    nc.sync.dma_start(
                        out=h_shard[m * 128:(m + 1) * 128, :], in_=hs[:]
                    )

            with nc.named_scope("allgather"):
                nc.gpsimd.collective_compute(
                    "AllGather",
                    mybir.AluOpType.bypass,
                    replica_groups=[list(range(N_CORES))],
                    ins=[h_shard[:]],
                    outs=[h_full[:]],
                )

            with nc.named_scope("edges"):
                for j in range(JT):
                    hs_g = gat.tile([128, D], bf16, name=f"hs_{j}", tag="hs")
                    hd_g = gat.tile([128, D], bf16, name=f"hd_{j}", tag="hd")
                    nc.gpsimd.indirect_dma_start(
                        out=hs_g[:],
                        out_offset=None,
                        in_=h_full[:],
                        in_offset=bass.IndirectOffsetOnAxis(
                            ap=sidx_sb[:, j:j + 1], axis=0
                        ),
                    )
                    nc.gpsimd.indirect_dma_start(
                        out=hd_g[:],
                        out_offset=None,
                        in_=h_full[:],
                        in_offset=bass.IndirectOffsetOnAxis(
                            ap=didx_sb[:, j:j + 1], axis=0
                        ),
                    )
                    prod = gat.tile([128, D], fp32, name=f"prod_{j}", tag="prod")
                    sq_s = gat.tile([128, D], fp32, name=f"sq_s_{j}", tag="sq_s")
                    sq_d = gat.tile([128, D], fp32, name=f"sq_d_{j}", tag="sq_d")
                    dot = small.tile([128, 1], fp32, name=f"dot_{j}", tag="dot")
                    ns = small.tile([128, 1], fp32, name=f"ns_{j}", tag="ns")
                    nd = small.tile([128, 1], fp32, name=f"nd_{j}", tag="nd")
                    # dot on DVE; squared norms on ACT (Square + row accum)
                    nc.vector.tensor_tensor(
                        out=prod[:], in0=hs_g[:], in1=hd_g[:], op=mybir.AluOpType.mult
                    )
                    nc.vector.tensor_reduce(
                        out=dot[:], in_=prod[:], axis=mybir.AxisListType.X,
                        op=mybir.AluOpType.add,
                    )
                    nc.scalar.activation(
                        sq_s[:], hs_g[:], mybir.ActivationFunctionType.Square,
                        accum_out=ns[:],
                    )
                    nc.scalar.activation(
                        sq_d[:], hd_g[:], mybir.ActivationFunctionType.Square,
                        accum_out=nd[:],
                    )
                    nsnd = small.tile([128, 1], fp32, name=f"nsnd_{j}", tag="nsnd")
                    nc.vector.tensor_tensor(
                        out=nsnd[:], in0=ns[:], in1=nd[:], op=mybir.AluOpType.mult
                    )
                    st = small.tile([128, 1], fp32, name=f"st_{j}", tag="st")
                    nc.scalar.sqrt(st[:], nsnd[:])
                    inv = small.tile([128, 1], fp32, name=f"inv_{j}", tag="inv")
                    nc.vector.reciprocal(inv[:], st[:])
                    ad = small.tile([128, 1], fp32, name=f"ad_{j}", tag="ad")
                    nc.scalar.activation(
                        ad[:], dot[:], mybir.ActivationFunctionType.Abs
                    )
                    nc.vector.tensor_tensor(
                        out=out_sb[:, j:j + 1],
                        in0=ad[:],
                        in1=inv[:],
                        op=mybir.AluOpType.mult,
                    )

            nc.sync.dma_start(out=out[:], in_=out_sb[:])

    nc.compile()
    return nc


def _get_nc():
    if "nc" not in _CACHE:
        _CACHE["nc"] = _build()
    return _CACHE["nc"]


def kernel(edges, A_s, emb, Ws, bs):
    global LAST_RESULTS
    from concourse.bass_utils import run_bass_kernel_spmd

    bf16 = ml_dtypes.bfloat16
    A = np.asarray(A_s, dtype=np.float32)
    E = np.asarray(emb, dtype=np.float32)
    W = np.asarray(Ws, dtype=np.float32)
    b = np.asarray(bs, dtype=np.float32)
    ed = np.asarray(edges)

    M = W[0].T @ W[1].T @ W[2].T                      # [D, D]
    # partition-major: [128(p), KT(t), D] with row t*128+p at [p, t, :]
    E2 = np.ascontiguousarray(
        (E @ M).astype(bf16).reshape(KT, 128, D).transpose(1, 0, 2)
    )
    b_eff = (b[0] @ W[1].T + b[1]) @ W[2].T + b[2]    # [D]
    bias_rep = np.ascontiguousarray(
        np.broadcast_to(b_eff.astype(np.float32), (128, D))
    )

    in_maps = []
    for c in range(N_CORES):
        at_c = np.ascontiguousarray(
            A[c * ROWS:(c + 1) * ROWS, :].T.astype(bf16)  # [N, ROWS]
            .reshape(KT, 128, ROWS).transpose(1, 0, 2)    # [128, KT, ROWS]
        )
        e = ed[c * EPC:(c + 1) * EPC]
        sidx_c = np.ascontiguousarray(e[:, 0].astype(np.int32).reshape(JT, 128).T)
        didx_c = np.ascontiguousarray(e[:, 1].astype(np.int32).reshape(JT, 128).T)
        in_maps.append(
            {"at": at_c, "e2": E2, "bias": bias_rep, "sidx": sidx_c, "didx": didx_c}
        )

    nc = _get_nc()
    res = run_bass_kernel_spmd(nc, in_maps, list(range(N_CORES)))
    LAST_RESULTS = res

    out = np.concatenate(
        [np.ascontiguousarray(res.results[c]["out"].T).reshape(-1) for c in range(N_CORES)]
    )
    return np.maximum(out, 0.0).astype(np.float32)


# revision 15
# speedup vs baseline: 1.1408x; 1.1408x over previous
"""Trainium2 Bass kernel for nn_MihGNNEmbeddingTest3 (gnn_message_passing).

Reference math:
    H = mlp(A_s @ emb)          (mlp = 3 linear layers, no activations)
    out[e] = relu(|<H[src_e], H[dst_e]>| / (||H[src_e]|| ||H[dst_e]||))

Since the mlp is affine, fold it:  H = A_s @ (emb @ W_eff^T) + b_eff.
Device work per core (node-sharded):  H_c = A_s[rows_c] @ E2 + b_eff
(E2 = emb @ W_eff^T precomputed on host), AllGather H, then per-edge
indirect row gathers + fused dot/norm reductions.

Sharding: A_s rows (and nodes) split 1024/core across 8 cores; edges
split 1024/core. A_s shard is shipped pre-transposed in bf16 so k-tiles
land directly as matmul lhsT weights.
"""

import os
import sys

import numpy as np

try:
    import concourse.bass  # noqa: F401
except ImportError:  # pragma: no cover - grading env should have PYTHONPATH set
    for p in ("/opt/trn_rl_repo", "/root/.axon_site/_ro/trn_rl_repo"):
        if os.path.isdir(p) and p not in sys.path:
            sys.path.insert(0, p)

import ml_dtypes

N, D, B = 8192, 256, 8192
N_CORES = 8
ROWS = N // N_CORES  # A_s rows / nodes per core
EPC = B // N_CORES   # edges per core
KT = N // 128        # contraction tiles
MT = ROWS // 128     # output row tiles per core
JT = EPC // 128      # edge blocks per core

_CACHE = {}
LAST_RESULTS = None  # BassKernelResults of the most recent run (for test.py)


def _build():
    import concourse.bacc as bacc
    import concourse.bass as bass
    import concourse.mybir as mybir
    import concourse.tile as tile

    fp32 = mybir.dt.float32
    bf16 = mybir.dt.bfloat16

    nc = bacc.Bacc(num_devices=N_CORES)
    # partition-major layouts: [p, k_tile, cols] so each DMA chunk reads
    # large contiguous per-partition spans from DRAM
    at = nc.declare_dram_parameter("at", [128, KT, ROWS], bf16, isOutput=False)
    e2 = nc.declare_dram_parameter("e2", [128, KT, D], bf16, isOutput=False)
    bias = nc.declare_dram_parameter("bias", [128, D], fp32, isOutput=False)
    sidx = nc.declare_dram_parameter("sidx", [128, JT], mybir.dt.int32, isOutput=False)
    didx = nc.declare_dram_parameter("didx", [128, JT], mybir.dt.int32, isOutput=False)
    out = nc.declare_dram_parameter("out", [128, JT], fp32, isOutput=True)

    with tile.TileContext(nc) as tc:
        with (
            tc.tile_pool(name="atp", bufs=1) as atp,
            tc.tile_pool(name="e2p", bufs=1) as e2p,
            tc.tile_pool(name="psum", bufs=MT, space="PSUM") as psum,
            tc.tile_pool(name="hsb", bufs=4) as hsbp,
            tc.tile_pool(name="dram", bufs=1, space="DRAM") as dram,
            tc.tile_pool(name="const", bufs=1) as constp,
            tc.tile_pool(name="gat", bufs=1) as gat,
            tc.tile_pool(name="small", bufs=1) as small,
        ):
            h_shard = dram.tile([ROWS, D], bf16)
            h_full = dram.tile([N, D], bf16, addr_space="Shared")

            # Batched loads: few big DMAs with 8-16KB contiguous descriptors
            # (per-dma_start issue overhead was pacing the whole matmul).
            # Small leading chunks so the first matmuls start early.
            AT_BOUNDS = [0, 2, 8, 16, 24, 32, 40, 48, 56, 64]
            E2_BOUNDS = [0, 4, 16, 32, 48, 64]
            at_t = [None] * KT
            e2_t = [None] * KT

            def load_e2(ci):
                lo, hi = E2_BOUNDS[ci], E2_BOUNDS[ci + 1]
                ec = e2p.tile([128, hi - lo, D], bf16, name=f"e2c_{ci}", tag=f"e2c{ci}")
                nc.sync.dma_start(out=ec[:], in_=e2[:, lo:hi, :])
                for k in range(lo, hi):
                    e2_t[k] = ec[:, k - lo, :]

            def load_at(ci):
                lo, hi = AT_BOUNDS[ci], AT_BOUNDS[ci + 1]
                ac = atp.tile([128, hi - lo, ROWS], bf16, name=f"atc_{ci}", tag=f"atc{ci}")
                nc.sync.dma_start(out=ac[:], in_=at[:, lo:hi, :])
                for k in range(lo, hi):
                    at_t[k] = ac[:, k - lo, :]

            # interleave so k-order data arrives first
            load_e2(0)
            load_at(0)
            load_at(1)
            load_e2(1)
            load_at(2)
            load_at(3)
            load_e2(2)
            load_at(4)
            load_at(5)
            load_e2(3)
            load_at(6)
            load_at(7)
            load_e2(4)
            load_at(8)

            bias_sb = constp.tile([128, D], fp32)
            nc.sync.dma_start(out=bias_sb[:], in_=bias[:])
            sidx_sb = constp.tile([128, JT], mybir.dt.int32)
            nc.sync.dma_start(out=sidx_sb[:], in_=sidx[:])
            didx_sb = constp.tile([128, JT], mybir.dt.int32)
            nc.sync.dma_start(out=didx_sb[:], in_=didx[:])
            out_sb = constp.tile([128, JT], fp32)

            with nc.named_scope("matmul"):
                # k-outer: every AT k-tile feeds all MT psum accumulators the
                # moment it lands, so PE keeps pace with the streaming DMA.
                ps_t = [
                    psum.tile([128, D], fp32, name=f"ps_{m}", tag="ps")
                    for m in range(MT)
                ]
                for k in range(KT):
                    for m in range(MT):
                        nc.tensor.matmul(
                            out=ps_t[m][:],
                            lhsT=at_t[k][:, m * 128:(m + 1) * 128],
                            rhs=e2_t[k],
                            start=(k == 0),
                            stop=(k == KT - 1),
                        )
                for m in range(MT):
                    hs = hsbp.tile([128, D], bf16, name=f"h_{m}", tag="h")
                    nc.vector.tensor_tensor(
                        out=hs[:], in0=ps_t[m][:], in1=bias_sb[:],
                        op=mybir.AluOpType.add,
                    )
                    nc.sync.dma_start(
                        out=h_shard[m * 128:(m + 1) * 128, :], in_=hs[:]
                    )

            with nc.named_scope("allgather"):
                nc.gpsimd.collective_compute(
                    "AllGather",
                    mybir.AluOpType.bypass,
                    replica_groups=[list(range(N_CORES))],
                    ins=[h_shard[:]],
                    outs=[h_full[:]],
                )

            with nc.named_scope("edges"):
                hs_all = gat.tile([128, JT, D], bf16, name="hs_all", tag="hs_all")
                hd_all = gat.tile([128, JT, D], bf16, name="hd_all", tag="hd_all")
                for j in range(JT):
                    nc.gpsimd.indirect_dma_start(
                        out=hs_all[:, j, :],
                        out_offset=None,
                        in_=h_full[:],
                        in_offset=bass.IndirectOffsetOnAxis(
                            ap=sidx_sb[:, j:j + 1], axis=0
                        ),
                    )
                    nc.gpsimd.indirect_dma_start(
                        out=hd_all[:, j, :],
                        out_offset=None,
                        in_=h_full[:],
                        in_offset=bass.IndirectOffsetOnAxis(
                            ap=didx_sb[:, j:j + 1], axis=0
                        ),
                    )
                # single-pass math over all JT blocks
                prod = gat.tile([128, JT, D], fp32, name="prod", tag="prod")
                sq_s = gat.tile([128, JT, D], fp32, name="sq_s", tag="sq_s")
                sq_d = gat.tile([128, JT, D], fp32, name="sq_d", tag="sq_d")
                dot = small.tile([128, JT], fp32, name="dot", tag="dot")
                ns = small.tile([128, JT], fp32, name="ns", tag="ns")
                nd = small.tile([128, JT], fp32, name="nd", tag="nd")
                nc.vector.tensor_tensor(
                    out=prod[:], in0=hs_all[:], in1=hd_all[:],
                    op=mybir.AluOpType.mult,
                )
                nc.vector.tensor_reduce(
                    out=dot[:], in_=prod[:], axis=mybir.AxisListType.X,
                    op=mybir.AluOpType.add,
                )
                nc.scalar.square(sq_s[:], hs_all[:])
                nc.scalar.square(sq_d[:], hd_all[:])
                nc.vector.tensor_reduce(
                    out=ns[:], in_=sq_s[:], axis=mybir.AxisListType.X,
                    op=mybir.AluOpType.add,
                )
                nc.vector.tensor_reduce(
                    out=nd[:], in_=sq_d[:], axis=mybir.AxisListType.X,
                    op=mybir.AluOpType.add,
                )
                nsnd = small.tile([128, JT], fp32, name="nsnd", tag="nsnd")
                nc.vector.tensor_tensor(
                    out=nsnd[:], in0=ns[:], in1=nd[:], op=mybir.AluOpType.mult
                )
                st = small.tile([128, JT], fp32, name="st", tag="st")
                nc.scalar.sqrt(st[:], nsnd[:])
                inv = small.tile([128, JT], fp32, name="inv", tag="inv")
                nc.vector.reciprocal(inv[:], st[:])
                ad = small.tile([128, JT], fp32, name="ad", tag="ad")
                nc.scalar.activation(
                    ad[:], dot[:], mybir.ActivationFunctionType.Abs
                )
                nc.vector.tensor_tensor(
                    out=out_sb[:],
                    in0=ad[:],
                    in1=inv[:],
                    op=mybir.AluOpType.mult,
                )

            nc.sync.dma_start(out=out[:], in_=out_sb[:])

    nc.compile()
    return nc


def _get_nc():
    if "nc" not in _CACHE:
        _CACHE["nc"] = _build()
    return _CACHE["nc"]


def kernel(edges, A_s, emb, Ws, bs):
    global LAST_RESULTS
    from concourse.bass_utils import run_bass_kernel_spmd

    bf16 = ml_dtypes.bfloat16
    A = np.asarray(A_s, dtype=np.float32)
    E = np.asarray(emb, dtype=np.float32)
    W = np.asarray(Ws, dtype=np.float32)
    b = np.asarray(bs, dtype=np.float32)
    ed = np.asarray(edges)

    M = W[0].T @ W[1].T @ W[2].T                      # [D, D]
    # partition-major: [128(p), KT(t), D] with row t*128+p at [p, t, :]
    E2 = np.ascontiguousarray(
        (E @ M).astype(bf16).reshape(KT, 128, D).transpose(1, 0, 2)
    )
    b_eff = (b[0] @ W[1].T + b[1]) @ W[2].T + b[2]    # [D]
    bias_rep = np.ascontiguousarray(
        np.broadcast_to(b_eff.astype(np.float32), (128, D))
    )

    in_maps = []
    for c in range(N_CORES):
        at_c = np.ascontiguousarray(
            A[c * ROWS:(c + 1) * ROWS, :].T.astype(bf16)  # [N, ROWS]
            .reshape(KT, 128, ROWS).transpose(1, 0, 2)    # [128, KT, ROWS]
        )
        e = ed[c * EPC:(c + 1) * EPC]
        sidx_c = np.ascontiguousarray(e[:, 0].astype(np.int32).reshape(JT, 128).T)
        didx_c = np.ascontiguousarray(e[:, 1].astype(np.int32).reshape(JT, 128).T)
        in_maps.append(
            {"at": at_c, "e2": E2, "bias": bias_rep, "sidx": sidx_c, "didx": didx_c}
        )

    nc = _get_nc()
    kw = {}
    if os.environ.get("KERNEL_TRACE_KW"):
        import json
        kw = json.loads(os.environ["KERNEL_TRACE_KW"])
    res = run_bass_kernel_spmd(nc, in_maps, list(range(N_CORES)), **kw)
    LAST_RESULTS = res

    out = np.concatenate(
        [np.ascontiguousarray(res.results[c]["out"].T).reshape(-1) for c in range(N_CORES)]
    )
    return np.maximum(out, 0.0).astype(np.float32)
